# revision 9
# baseline (speedup 1.0000x reference)
"""Edge-parallel ExtractorMLP (gather + 3-layer MLP) for 8 TRN2 NeuronCores.

Strategy (pure edge parallelism, no cross-core communication):
  - All 800K edges are sorted globally by (row_half, col) and dealt
    round-robin to the 8 cores, so every core's tile t draws its edges
    from the same 4096-edge window of the global sort. Tile metadata
    (row table half, col chunk window) is therefore identical across
    cores and can be baked into the single SPMD program.
  - COL endpoint: because cols are sorted, a 512-edge tile's cols span
    ~512 consecutive nodes (~5 aligned 128-node chunks). The gather is
    done ON THE TENSOR ENGINE as one-hot matmuls: a node-major copy of
    the embedding table lives in SBUF ([128 node partitions x 391
    chunks x 128 features]); for each chunk a [128, 512] 0/1 selection
    matrix S (built by the vector engine from DMA-broadcast col values
    via subtract+is_equal against a per-partition iota) is multiplied
    against the chunk to accumulate emb[col] in PSUM - exact, and it
    rides otherwise-idle PE/DVE cycles.
  - ROW endpoint: rows are random, so they use SWDGE dma_gather
    (transpose=True) from the HBM [50000, 128] fp16 table: 512 indices
    per tile, ~9.3ns/descriptor of Q7 time - the pacing engine. Rows
    are int16 per dma_gather's ABI, hence the row_half split (<32768
    nodes per half, half-local indices).
  - The MLP runs per 512-edge tile on the tensor engine in fp16 with
    fp32 PSUM accumulation: layer 1 as 4 M-chunks x 2 K-chunks (K-chunk
    0 is the one-hot col gather, chunk 1 the row gather), layer 2 as 4
    K-chunks, layer 3 as a single [128,1] stationary matmul. Bias+ReLU
    epilogues are split between the scalar (ACT) and vector (DVE)
    engines; col-value broadcasts and S-builds are software-pipelined
    one to two tiles ahead so no engine queue blocks another.
  - Edge order is restored on the host afterwards.
"""

from contextlib import ExitStack

import numpy as np

import concourse.bacc as bacc
import concourse.tile as tile
from concourse import mybir
from concourse.bass_utils import run_bass_kernel_spmd

P = 128
N = 512            # edges per tile (one fp32 PSUM bank)
IDXW = N // 16     # wrapped-index columns per tile
N_CORES = 8
GT = N * N_CORES   # global edges per tile row (4096)
N_NODES = 50000
N_NODES_PAD = 50048  # 391 chunks of 128
NCH_TBL = N_NODES_PAD // 128
N_EDGES = 800000
E_CORE = N_EDGES // N_CORES
HALF = 25000       # row table half size (int16 dma_gather indices)

F16 = mybir.dt.float16
F32 = mybir.dt.float32
I16 = mybir.dt.int16


def _build_kernel(tiles_meta: tuple):
    """tiles_meta: per-tile (row_half, col_chunk_lo, n_chunks), compile-time."""
    nc = bacc.Bacc("TRN2", target_bir_lowering=False, debug=False)
    n_tiles = len(tiles_meta)

    tblrow = nc.dram_tensor("tblrow", [N_NODES, P], F16, kind="ExternalInput")
    tblnm = nc.dram_tensor("tblnm", [P, NCH_TBL * 128], F16, kind="ExternalInput")
    roww = nc.dram_tensor("roww", [P, n_tiles * IDXW], I16, kind="ExternalInput")
    colloc = nc.dram_tensor("colloc", [1, n_tiles * N], F16, kind="ExternalInput")
    iota = nc.dram_tensor("iota", [P, 1], F32, kind="ExternalInput")
    w1 = nc.dram_tensor("w1", [P, 1024], F16, kind="ExternalInput")
    w2 = nc.dram_tensor("w2", [P, 512], F16, kind="ExternalInput")
    w3 = nc.dram_tensor("w3", [P, 1], F16, kind="ExternalInput")
    b1 = nc.dram_tensor("b1", [P, 4], F32, kind="ExternalInput")
    b2 = nc.dram_tensor("b2", [P, 1], F32, kind="ExternalInput")
    b3 = nc.dram_tensor("b3", [1, 1], F32, kind="ExternalInput")
    out = nc.dram_tensor("out", [n_tiles, N], F32, kind="ExternalOutput")

    Relu = mybir.ActivationFunctionType.Relu
    Identity = mybir.ActivationFunctionType.Identity
    Op = mybir.AluOpType

    with tile.TileContext(nc) as tc, ExitStack() as ctx:
        tp = ctx.enter_context(tc.tile_pool(name="tp", bufs=1))
        idxp = ctx.enter_context(tc.tile_pool(name="idxp", bufs=1))
        wp = ctx.enter_context(tc.tile_pool(name="wp", bufs=1))
        cbp = ctx.enter_context(tc.tile_pool(name="cbp", bufs=4))
        sp = ctx.enter_context(tc.tile_pool(name="sp", bufs=14))
        grp = ctx.enter_context(tc.tile_pool(name="grp", bufs=6))
        gcp = ctx.enter_context(tc.tile_pool(name="gcp", bufs=4))
        x1p = ctx.enter_context(tc.tile_pool(name="x1p", bufs=12))
        x2p = ctx.enter_context(tc.tile_pool(name="x2p", bufs=4))
        op = ctx.enter_context(tc.tile_pool(name="op", bufs=8))
        pg = ctx.enter_context(tc.tile_pool(name="pg", bufs=2, space="PSUM"))
        pl1 = ctx.enter_context(tc.tile_pool(name="pl1", bufs=4, space="PSUM"))
        pl2 = ctx.enter_context(tc.tile_pool(name="pl2", bufs=1, space="PSUM"))
        pl3 = ctx.enter_context(tc.tile_pool(name="pl3", bufs=1, space="PSUM"))

        # ---- one-time loads -------------------------------------------
        tblnm_sb = tp.tile([P, NCH_TBL * 128], F16)
        n_dma = 16
        cs = (NCH_TBL * 128 + n_dma - 1) // n_dma
        for c in range(n_dma):
            lo, hi = c * cs, min((c + 1) * cs, NCH_TBL * 128)
            nc.sync.dma_start(tblnm_sb[:, lo:hi], tblnm[:, lo:hi])

        roww_sb = idxp.tile([P, n_tiles * IDXW], I16)
        nc.scalar.dma_start(roww_sb[:], roww[:])
        iota_sb = wp.tile([P, 1], F32)
        nc.scalar.dma_start(iota_sb[:], iota[:])

        w1_sb = wp.tile([P, 1024], F16)
        w2_sb = wp.tile([P, 512], F16)
        w3_sb = wp.tile([P, 1], F16)
        b1_sb = wp.tile([P, 4], F32)
        b2_sb = wp.tile([P, 1], F32)
        b3_sb = wp.tile([1, 1], F32)
        nc.scalar.dma_start(w1_sb[:], w1[:])
        nc.scalar.dma_start(w2_sb[:], w2[:])
        nc.scalar.dma_start(w3_sb[:], w3[:])
        nc.scalar.dma_start(b1_sb[:], b1[:])
        nc.scalar.dma_start(b2_sb[:], b2[:])
        nc.scalar.dma_start(b3_sb[:], b3[:])

        # col values broadcast (scalar HWDGE) and one-hot S builds (DVE)
        # are software-pipelined ahead of their consuming tile.
        def emit_cb(t):
            cb = cbp.tile([P, N], F16, tag="cb", name=f"cb{t}")
            nc.sync.dma_start(
                cb[:], colloc[0:1, t * N:(t + 1) * N].broadcast_to([P, N]))
            return cb

        def emit_s(t, cb):
            nch = tiles_meta[t][2]
            ss = []
            for kk in range(nch):
                s = sp.tile([P, N], F16, tag="S", name=f"s{t}_{kk}")
                nc.vector.tensor_scalar(
                    out=s[:], in0=cb[:], scalar1=iota_sb[:, 0:1],
                    scalar2=float(128 * kk),
                    op0=Op.subtract, op1=Op.is_equal,
                )
                ss.append(s)
            return ss

        cbs = {0: emit_cb(0)}
        if n_tiles > 1:
            cbs[1] = emit_cb(1)
        s_next = emit_s(0, cbs[0])

        # ---- steady state ---------------------------------------------
        for t, (rh, clo, nch) in enumerate(tiles_meta):
            # row endpoint: SWDGE gather from HBM (feature-major output)
            g_row = grp.tile([P, 1, N], F16, tag="grow")
            nc.gpsimd.dma_gather(
                g_row[:], tblrow[rh * HALF:rh * HALF + HALF, :],
                roww_sb[:, t * IDXW:(t + 1) * IDXW], N, N, P, transpose=True,
            )

            if t + 2 < n_tiles:
                cbs[t + 2] = emit_cb(t + 2)

            # col endpoint: one-hot matmuls against node-major table chunks
            s_cur = s_next
            pg_t = pg.tile([P, N], F32, tag="pg")
            for kk in range(nch):
                nc.tensor.matmul(
                    pg_t[:],
                    lhsT=tblnm_sb[:, (clo + kk) * 128:(clo + kk + 1) * 128],
                    rhs=s_cur[kk][:], start=(kk == 0), stop=(kk == nch - 1),
                )
            g_col = gcp.tile([P, N], F16, tag="gcol")
            nc.vector.tensor_scalar(
                out=g_col[:], in0=pg_t[:], scalar1=0.0, scalar2=None,
                op0=Op.add,
            )

            if t + 1 < n_tiles:
                s_next = emit_s(t + 1, cbs[t + 1])

            # layer 1: [E,256] @ [256,512]; K-chunk 0 = col, 1 = row
            x1s = []
            for m in range(4):
                p1 = pl1.tile([P, N], F32, tag="pl1")
                nc.tensor.matmul(
                    p1[:], lhsT=w1_sb[:, m * 128:(m + 1) * 128],
                    rhs=g_col[:], start=True, stop=False,
                )
                nc.tensor.matmul(
                    p1[:], lhsT=w1_sb[:, 512 + m * 128: 512 + (m + 1) * 128],
                    rhs=g_row[:, 0, :], start=False, stop=True,
                )
                x1 = x1p.tile([P, N], F16, tag="x1")
                if m < 3:
                    nc.scalar.activation(
                        x1[:], p1[:], Relu, bias=b1_sb[:, m:m + 1]
                    )
                else:
                    nc.vector.tensor_scalar(
                        out=x1[:], in0=p1[:],
                        scalar1=b1_sb[:, m:m + 1], scalar2=0.0,
                        op0=Op.add, op1=Op.max,
                    )
                x1s.append(x1)

            # layer 2: [E,512] @ [512,128]
            p2 = pl2.tile([P, N], F32, tag="pl2")
            for k in range(4):
                nc.tensor.matmul(
                    p2[:], lhsT=w2_sb[:, k * 128:(k + 1) * 128],
                    rhs=x1s[k][:], start=(k == 0), stop=(k == 3),
                )
            x2 = x2p.tile([P, N], F16, tag="x2")
            nc.scalar.activation(x2[:], p2[:], Relu, bias=b2_sb[:, 0:1])

            # layer 3: [E,128] @ [128,1]
            p3 = pl3.tile([P, N], F32, tag="pl3")
            nc.tensor.matmul(p3[:1, :], lhsT=w3_sb[:], rhs=x2[:],
                             start=True, stop=True)
            o = op.tile([1, N], F32, tag="o")
            nc.scalar.activation(o[:1, :], p3[:1, :], Identity,
                                 bias=b3_sb[:1, 0:1])
            nc.sync.dma_start(out[t:t + 1, :], o[:])

    nc.compile()
    return nc


def _wrap_indices(idx: np.ndarray) -> np.ndarray:
    """[n_tiles*512] local ids -> [128, n_tiles*32] int16 wrapped layout.

    dma_gather unwraps each 16-partition group as
    rearrange("p s -> (s p)"), so index j of tile t sits at
    [16g + j%16, t*32 + j//16], replicated over the 8 groups g.
    """
    n_tiles = idx.shape[0] // N
    w = idx.astype(np.int16).reshape(n_tiles, IDXW, 16).transpose(0, 2, 1)
    w = np.tile(w, (1, 8, 1))
    return np.ascontiguousarray(w.transpose(1, 0, 2).reshape(P, n_tiles * IDXW))


def _plan(edge_index):
    """Global (row_half, col) sort + round-robin deal to cores.

    Returns (tiles_meta, per-core (colloc f16 [1, S], row_local i64 [S],
    slot_orig i64 [S])) with S = n_tiles*512 slots per core.
    """
    col = np.asarray(edge_index[0], dtype=np.int64)
    row = np.asarray(edge_index[1], dtype=np.int64)
    half = (row >= HALF).astype(np.int64)
    order = np.lexsort((col, half))
    scol, srow, shalf = col[order], row[order], half[order]
    b0 = int((half == 0).sum())
    bounds = [(0, b0, 0), (b0, N_EDGES, 1)]

    tiles_meta = []
    # padded global slot -> sorted-position (or -1)
    gslots = []
    for s, e, k in bounds:
        nt = -(-(e - s) // GT)
        for i in range(nt):
            p0, p1 = s + i * GT, min(s + (i + 1) * GT, e)
            wlo = int(scol[p0])
            whi = int(scol[p1 - 1])
            clo = wlo >> 7
            nch = (whi >> 7) - clo + 1
            tiles_meta.append((k, clo, nch))
            sl = np.full(GT, -1, np.int64)
            sl[:p1 - p0] = np.arange(p0, p1)
            gslots.append(sl)
    g = np.stack(gslots)                      # [n_tiles, GT]
    n_tiles = len(tiles_meta)
    g = g.reshape(n_tiles, N, N_CORES)        # [t, j, core]

    clo_arr = np.array([m[1] for m in tiles_meta], np.int64)[:, None]
    rh_arr = np.array([m[0] for m in tiles_meta], np.int64)[:, None]

    per_core = []
    for c in range(N_CORES):
        gp = g[:, :, c]                       # [t, j] sorted positions
        valid = gp >= 0
        gp_safe = np.where(valid, gp, 0)
        cl = np.where(valid, scol[gp_safe] - (clo_arr << 7), 0)
        rl = np.where(valid, srow[gp_safe] - rh_arr * HALF, 0)
        so = np.where(valid, order[gp_safe], -1)
        assert cl.max() < 2048, cl.max()
        per_core.append((
            cl.reshape(-1).astype(np.float16)[None, :],
            rl.reshape(-1),
            so.reshape(-1),
        ))
    return tuple(tiles_meta), per_core


def _prep_shared(emb, W1, b1, W2, b2, W3, b3):
    emb16 = emb.astype(np.float16)
    pad = np.zeros((N_NODES_PAD, P), np.float16)
    pad[:N_NODES] = emb16
    tblnm = np.ascontiguousarray(
        pad.reshape(NCH_TBL, 128, 128).transpose(1, 0, 2).reshape(P, -1))
    return {
        "tblrow": np.ascontiguousarray(emb16),
        "tblnm": tblnm,
        "iota": np.arange(128, dtype=np.float32)[:, None],
        "w1": np.ascontiguousarray(
            np.concatenate([W1[:128, :], W1[128:, :]], axis=1)
        ).astype(np.float16),
        "w2": np.ascontiguousarray(
            np.concatenate([W2[k * 128:(k + 1) * 128, :] for k in range(4)],
                           axis=1)
        ).astype(np.float16),
        "w3": W3.astype(np.float16),
        "b1": np.ascontiguousarray(b1.reshape(4, 128).T).astype(np.float32),
        "b2": b2[:, None].astype(np.float32),
        "b3": b3[None, :].astype(np.float32),
    }


_NC_CACHE = {}


def _get_nc(tiles_meta):
    if tiles_meta not in _NC_CACHE:
        _NC_CACHE[tiles_meta] = _build_kernel(tiles_meta)
    return _NC_CACHE[tiles_meta]


def run(inputs: dict, trace: bool = False):
    """Run the kernel on 8 cores; returns (out [800000,1] f32, results)."""
    emb = np.asarray(inputs["emb"], dtype=np.float32)
    edge_index = np.asarray(inputs["edge_index"])
    shared = _prep_shared(
        emb,
        *[np.asarray(inputs[k], dtype=np.float32)
          for k in ("W1", "b1", "W2", "b2", "W3", "b3")]
    )
    tiles_meta, per_core = _plan(edge_index)
    in_maps = [
        dict(shared, colloc=np.ascontiguousarray(cl),
             roww=_wrap_indices(rl))
        for (cl, rl, _) in per_core
    ]
    nc = _get_nc(tiles_meta)
    res = run_bass_kernel_spmd(nc, in_maps, list(range(N_CORES)), trace=trace)
    out = np.empty((N_EDGES,), np.float32)
    for c in range(N_CORES):
        flat = res.results[c]["out"].reshape(-1)
        so = per_core[c][2]
        valid = so >= 0
        out[so[valid]] = flat[valid]
    return out[:, None], res


def kernel(**inputs) -> np.ndarray:
    out, _ = run(inputs, trace=False)
    return out
